# revision 9
# baseline (speedup 1.0000x reference)
"""GAT (graph attention) Bass kernel for Trainium2, 8-core SPMD — v3.

Strategy: receiver-per-partition windows + batched dma_gather.

v2 issued one indirect DMA per 128-row block (~900 calls/core x ~1us
SWDGE fixed cost = the whole phase-B budget). v3 packs each chunk's
receiver+sender rows into ONE InstDMAGatherAnt call (14 calls/core).

dma_gather indices are int16 (max 32767), so the node table stores
PAIRS of h-vectors: row j = [h(2j) | h(2j+1)] fp16 (256B), 25088 pairs
+ 1 zero sentinel < 32767. Attention scores are no longer precomputed
in the table; phase B computes s1/s2 for BOTH pair halves from the
gathered row (a1cat/a2cat elementwise + 16-group reduce) and a host-
built mask stream adds 0 / -100 per (slot, half): the wrong half and
pad slots exp() to exactly 0 in fp16, so numerator (128-col) and
denominator (8-col) reductions just fold A+B halves at the end.

Per core:
  phase A: tab2[j] = [h(2j)(64) | h(2j+1)(64)] fp16, written via the
           same block-permuted contiguous stores as v2 (gather indices
           absorb the permutation); one zero sentinel row at `pairs`.
  phase B: per chunk (<=8 windows, <=64 sender blocks): one dma_gather
           fetches nwin receiver blocks + nb sender blocks (128 rows
           each) from tab2; DVE/ACT compute per window; output DMA.

Host scatters the staged [128, 64] window outputs back to node order.
"""

import os
import sys

import numpy as np

for _p in ("/opt/trn_rl_repo", os.path.expanduser("~/.axon_site/_ro/trn_rl_repo")):
    if os.path.isdir(_p) and _p not in sys.path:
        sys.path.insert(0, _p)

P = 128
XTILE = 1024                 # phase-A node super-tile
NBLK = XTILE // P            # 8
HEADS = 4
UNITS = 16
HU = HEADS * UNITS           # 64
PAIRC = 2 * HU               # 128 fp16 cols = 256B pair row
LEAKY_ALPHA = 0.2
CSHIFT = 3.5                 # global exp shift (softmax-invariant)
PEN = -100.0                 # per-(slot,half) penalty => exp==0 in fp16
BCAP = 64                    # max sender blocks per gather chunk
WCAP = 8                     # max windows per gather chunk
QSPLIT = 2                   # SWDGE queues (2 Q7 desc-gen contexts run in parallel)
ABLATE = "full"              # dev-only: "phaseA" | "nocompute"
REPS = 1                     # dev-only: replicate kernel body for timing
CLEVEL = 6                   # dev-only: compute-stage ablation (0..6)


def _pair(n):
    """Node id -> (pair row, half) in the block-permuted pair table."""
    n = np.asarray(n)
    t, r = n // XTILE, n % XTILE
    p, i = r % P, r // P
    return t * (XTILE // 2) + p * (NBLK // 2) + i // 2, i % 2


def _build_host_data(x, edge_index, W, att_w1, att_w2, n_cores):
    n_nodes, in_feat = x.shape
    snd = edge_index[:, 0].astype(np.int64)
    rcv = edge_index[:, 1].astype(np.int64)

    ntiles = -(-n_nodes // XTILE)
    npad = ntiles * XTILE
    pairs = npad // 2
    sent = pairs  # sentinel pair row (zeros)

    deg = np.bincount(rcv, minlength=n_nodes)
    active = np.nonzero(deg > 0)[0]
    order_n = active[np.argsort(deg[active], kind="stable")]

    wtot = -(-len(order_n) // P)
    nw = -(-wtot // n_cores)
    wpad = nw * n_cores
    win_nodes_g = np.full((wpad, P), -1, dtype=np.int64)
    win_nodes_g.reshape(-1)[: len(order_n)] = order_n

    deg_g = np.where(win_nodes_g >= 0, deg[win_nodes_g], 0)
    k_g = deg_g.max(axis=1)
    # per-local-window block cap: max over the n_cores interleaved windows
    K = k_g.reshape(nw, n_cores).max(axis=1).astype(np.int64)

    # chunking: greedy, <= BCAP sender blocks and <= WCAP windows per chunk
    chunks = []  # list of (w0, nwin)
    w = 0
    while w < nw:
        w0 = w
        blocks = 0
        while w < nw and (w - w0) < WCAP and (blocks + K[w]) <= max(BCAP, K[w]):
            blocks += K[w]
            w += 1
        chunks.append((w0, w - w0))

    base = np.zeros(nw + 1, dtype=np.int64)
    base[1:] = np.cumsum(K)
    btot = int(base[-1])
    TB = nw + btot  # total stream blocks per core

    # stream columns: per chunk [recv blocks | sender blocks]
    rcol = np.zeros(nw, dtype=np.int64)   # stream block of window w's recv row
    scol = np.zeros(nw, dtype=np.int64)   # stream block of window w's 1st sender
    for (w0, nwin) in chunks:
        S = w0 + base[w0]
        for i in range(nwin):
            rcol[w0 + i] = S + i
            scol[w0 + i] = S + nwin + (base[w0 + i] - base[w0])

    # node -> (core, local w, partition)
    node_c = np.full(n_nodes, -1, dtype=np.int64)
    node_w = np.zeros(n_nodes, dtype=np.int64)
    node_p = np.zeros(n_nodes, dtype=np.int64)
    gwin = np.repeat(np.arange(wpad), P).reshape(wpad, P)
    valid = win_nodes_g >= 0
    vn = win_nodes_g[valid]
    node_c[vn] = gwin[valid] % n_cores
    node_w[vn] = gwin[valid] // n_cores
    node_p[vn] = np.tile(np.arange(P), wpad).reshape(wpad, P)[valid]

    # edge -> slot k within its receiver's run
    eorder = np.argsort(rcv, kind="stable")
    rs = rcv[eorder]
    ss = snd[eorder]
    starts = np.zeros(n_nodes + 1, dtype=np.int64)
    starts[1:] = np.cumsum(deg)
    k_e = np.arange(len(rs)) - starts[rs]
    pair_ss, half_ss = _pair(ss)

    xT16 = np.zeros((in_feat, npad), dtype=np.float16)
    xT16[:, :n_nodes] = np.ascontiguousarray(x.T).astype(np.float16)

    w16 = W.astype(np.float16)  # [in_feat, 64]

    # a1cat/a2cat: [128] = per-pair-col attention vector, replicated to 128
    # partitions -> aa [128, 256] = [a1cat | a2cat]
    a1 = att_w1.reshape(HEADS, UNITS).astype(np.float16)  # [4, 16]
    a2 = att_w2.reshape(HEADS, UNITS).astype(np.float16)
    a1cat = np.concatenate([a1.reshape(-1), a1.reshape(-1)])  # [128]
    a2cat = np.concatenate([a2.reshape(-1), a2.reshape(-1)])
    aa = np.tile(np.concatenate([a1cat, a2cat])[None, :], (P, 1))  # [128, 256]

    per_core = []
    win_nodes_c_all = []
    for c in range(n_cores):
        wn = win_nodes_g[c::n_cores]  # [nw, 128]
        emask = node_c[rs] == c
        er = rs[emask]
        ew = node_w[er]
        ep = node_p[er]
        ek = k_e[emask]

        # flat stream position j = B*128 + p
        idxf = np.full(TB * P, sent, dtype=np.int32)
        half = np.zeros(TB * P, dtype=np.int64)
        kind = np.zeros(TB * P, dtype=np.int8)  # 0=pad, 1=sender, 2=recv

        j_e = (scol[ew] + ek) * P + ep
        idxf[j_e] = pair_ss[emask]
        half[j_e] = half_ss[emask]
        kind[j_e] = 1

        rvalid = wn >= 0
        pr, hr = _pair(np.maximum(wn, 0))
        j_r = rcol[:, None] * P + np.arange(P)[None, :]  # [nw, 128]
        idxf[j_r.reshape(-1)] = np.where(rvalid, pr, sent).reshape(-1)
        half[j_r.reshape(-1)] = np.where(rvalid, hr, 0).reshape(-1)
        kind[j_r.reshape(-1)] = np.where(rvalid, 2, 0).reshape(-1)

        # mask stream [TB*P, 8]: senders get {0, PEN} penalties per half,
        # receivers get {1, 0} selector bits per half, pads get PEN.
        mask = np.zeros((TB * P, 8), dtype=np.float16)
        isA = half == 0
        s = kind == 1
        mask[s, 0:4] = np.where(isA[s, None], 0.0, PEN)
        mask[s, 4:8] = np.where(isA[s, None], PEN, 0.0)
        r = kind == 2
        mask[r, 0:4] = np.where(isA[r, None], 1.0, 0.0)
        mask[r, 4:8] = np.where(isA[r, None], 0.0, 1.0)
        pad = kind == 0
        mask[pad, :] = PEN

        # idx16: wrapped-16 layout [16, TB*8] replicated to [128, TB*8]
        idxw = idxf.astype(np.int16).reshape(TB * 8, 16).T  # [16, TB*8]
        idx16 = np.ascontiguousarray(np.tile(idxw, (8, 1)))  # [128, TB*8]

        mask16 = np.ascontiguousarray(
            mask.reshape(TB, P, 8).transpose(1, 0, 2).reshape(P, TB * 8))

        per_core.append({
            "xT16": xT16,
            "w16": w16,
            "aa": aa,
            "idx16": idx16,
            "mask16": mask16,
        })
        win_nodes_c_all.append(wn)

    plan = {
        "npad": npad, "ntiles": ntiles, "nw": nw, "pairs": pairs,
        "K": K.tolist(), "base": base.tolist(), "btot": btot, "TB": TB,
        "chunks": chunks, "in_feat": in_feat,
    }
    host = {"plan": plan, "win_nodes": win_nodes_c_all, "n_nodes": n_nodes}
    return host, per_core


def _build_bass(plan):
    from concourse import bacc, mybir, tile
    import concourse.bass as bass

    f16 = mybir.dt.float16
    f32 = mybir.dt.float32
    i16 = mybir.dt.int16

    npad = plan["npad"]
    ntiles = plan["ntiles"]
    nw = plan["nw"]
    pairs = plan["pairs"]
    K = plan["K"]
    base = plan["base"]
    TB = plan["TB"]
    chunks = plan["chunks"]
    in_feat = plan["in_feat"]

    nc = bacc.Bacc("TRN2", target_bir_lowering=False, debug=False,
                   enable_asserts=False, num_devices=1,
                   num_swdge_queues=QSPLIT)

    xT_d = nc.dram_tensor("xT16", [in_feat, npad], f16, kind="ExternalInput").ap()
    w_d = nc.dram_tensor("w16", [in_feat, HU], f16, kind="ExternalInput").ap()
    aa_d = nc.dram_tensor("aa", [P, 2 * PAIRC], f16, kind="ExternalInput").ap()
    idx_d = nc.dram_tensor("idx16", [P, TB * 8], i16, kind="ExternalInput").ap()
    mask_d = nc.dram_tensor("mask16", [P, TB * 8], f16, kind="ExternalInput").ap()

    out_d = nc.dram_tensor("staged", [nw * P, HU], f32, kind="ExternalOutput").ap()
    tab_d = nc.dram_tensor("tab2", [pairs + 1, PAIRC], f16, kind="Internal").ap()

    with tile.TileContext(nc) as tc:
        with tc.tile_pool(name="consts", bufs=1) as cpool:
            w_sb = cpool.tile([in_feat, HU], f16, tag="w16")
            nc.sync.dma_start(out=w_sb[:], in_=w_d[:])
            aa_sb = cpool.tile([P, 2 * PAIRC], f16, tag="aa")
            nc.sync.dma_start(out=aa_sb[:], in_=aa_d[:])
            idx_sb = cpool.tile([P, TB * 8], i16, tag="idx16")
            nc.sync.dma_start(out=idx_sb[:], in_=idx_d[:])
            mask_sb = cpool.tile([P, TB * 8], f16, tag="mask16")
            nc.sync.dma_start(out=mask_sb[:], in_=mask_d[:])
            zrow = cpool.tile([1, PAIRC], f16, tag="zrow")
            nc.gpsimd.memset(zrow[:], 0.0)
            nc.sync.dma_start(out=tab_d[pairs:pairs + 1, :], in_=zrow[:])
            cbias = cpool.tile([P, 1], f32, tag="cbias")
            nc.gpsimd.memset(cbias[:], -CSHIFT)

            # ---- phase A: pair table tab2[t*512 + p*4 + i] ----
            with tc.tile_pool(name="pa_x", bufs=3) as pax, \
                 tc.tile_pool(name="pa_ps", bufs=4, space="PSUM") as paps, \
                 tc.tile_pool(name="pa_hs", bufs=3) as pahs:
              for _rep in range(REPS):
                for t in range(ntiles):
                    xt = pax.tile([in_feat, XTILE], f16, tag="xt")
                    nc.sync.dma_start(
                        out=xt[:], in_=xT_d[:, t * XTILE:(t + 1) * XTILE])
                    hst = pahs.tile([P, NBLK * HU], f16, tag="hst")
                    half = NBLK // 2
                    for g in range(2):
                        ps = paps.tile([P, half * HU], f32, tag="ps")
                        for i in range(half):
                            b = g * half + i
                            nc.tensor.matmul(
                                out=ps[:, i * HU:(i + 1) * HU],
                                lhsT=xt[:, b * P:(b + 1) * P],
                                rhs=w_sb[:], start=True, stop=True)
                        nc.vector.tensor_copy(
                            out=hst[:, g * half * HU:(g + 1) * half * HU],
                            in_=ps[:])
                    nc.sync.dma_start(
                        out=tab_d[t * (XTILE // 2):(t + 1) * (XTILE // 2),
                                  :].rearrange("(p i) c -> p i c", p=P),
                        in_=hst[:].rearrange("p (i c) -> p i c", c=PAIRC))

            # ---- phase B: chunked gather + chunk-wide attention ----
            if ABLATE != "phaseA":
              with tc.tile_pool(name="pb_hs", bufs=2) as pbh, \
                   tc.tile_pool(name="pb_w", bufs=2) as pbw, \
                   tc.tile_pool(name="pb_o", bufs=2) as pbo:
                for _rep in range(REPS):
                  for _ci, (w0, nwin) in enumerate(chunks):
                    nb = sum(K[w0:w0 + nwin])
                    S = w0 + base[w0]
                    nbt = nwin + nb
                    hs = pbh.tile([P, nbt * PAIRC], f16, tag="hs")
                    hs3 = hs[:].rearrange("p (j c) -> p j c", c=PAIRC)
                    nc.gpsimd.dma_gather(
                        out_ap=hs3,
                        in_ap=tab_d[:],
                        idxs_ap=idx_sb[:, S * 8:(S + nbt) * 8],
                        num_idxs=nbt * P,
                        num_idxs_reg=nbt * P,
                        elem_size=PAIRC,
                        single_packet=False,
                        queue_num=_ci % QSPLIT)
                    if ABLATE == "nocompute":
                        continue

                    osb_c = pbo.tile([P, nwin * HU], f32, tag="osb")
                    osb3 = osb_c[:].rearrange("p (i c) -> p i c", c=HU)

                    if CLEVEL >= 1:
                        # receiver s2: both halves, chunk-wide
                        s2m = pbw.tile([P, nwin * PAIRC], f16, tag="s2m")
                        nc.vector.tensor_tensor(
                            out=s2m[:].rearrange("p (j c) -> p j c", c=PAIRC),
                            in0=hs3[:, 0:nwin, :],
                            in1=aa_sb[:, PAIRC:2 * PAIRC].unsqueeze(
                                1).broadcast_to([P, nwin, PAIRC]),
                            op=mybir.AluOpType.mult)
                        s2b = pbw.tile([P, nwin * 8], f16, tag="s2b")
                        with nc.allow_low_precision(reason="fp16 logits"):
                            nc.vector.tensor_reduce(
                                out=s2b[:],
                                in_=s2m[:].rearrange("p (a u) -> p a u",
                                                     u=UNITS),
                                axis=mybir.AxisListType.X,
                                op=mybir.AluOpType.add)
                        s2s = pbw.tile([P, nwin * 8], f16, tag="s2s")
                        nc.vector.tensor_tensor(
                            out=s2s[:], in0=s2b[:],
                            in1=mask_sb[:, S * 8:(S + nwin) * 8],
                            op=mybir.AluOpType.mult)
                        s2s3 = s2s[:].rearrange("p (i t h) -> p i t h",
                                                t=2, h=HEADS)
                        s2f = pbw.tile([P, nwin * HEADS], f16, tag="s2f")
                        nc.vector.tensor_tensor(
                            out=s2f[:].rearrange("p (i h) -> p i h", h=HEADS),
                            in0=s2s3[:, :, 0, :], in1=s2s3[:, :, 1, :],
                            op=mybir.AluOpType.add)
                        s2c8 = pbw.tile([P, nwin * 8], f16, tag="s2c8")
                        s2c83 = s2c8[:].rearrange("p (i q) -> p i q", q=8)
                        s2f3 = s2f[:].rearrange("p (i h) -> p i h", h=HEADS)
                        nc.vector.tensor_copy(out=s2c83[:, :, 0:4], in_=s2f3)
                        nc.vector.tensor_copy(out=s2c83[:, :, 4:8], in_=s2f3)
                        # expand per window to k-repeats
                        s2str = pbw.tile([P, nb * 8], f16, tag="s2str")
                        s2str3 = s2str[:].rearrange("p (j q) -> p j q", q=8)
                        off = 0
                        for i in range(nwin):
                            k = K[w0 + i]
                            if k == 0:
                                continue
                            nc.vector.tensor_copy(
                                out=s2str3[:, off:off + k, :],
                                in_=s2c8[:, i * 8:(i + 1) * 8].unsqueeze(
                                    1).broadcast_to([P, k, 8]))
                            off += k

                    if CLEVEL >= 2:
                        # sender s1: both halves, chunk-wide
                        s1m = pbw.tile([P, nb * PAIRC], f16, tag="s1m")
                        nc.vector.tensor_tensor(
                            out=s1m[:].rearrange("p (j c) -> p j c", c=PAIRC),
                            in0=hs3[:, nwin:nbt, :],
                            in1=aa_sb[:, 0:PAIRC].unsqueeze(1).broadcast_to(
                                [P, nb, PAIRC]),
                            op=mybir.AluOpType.mult)
                        s1b = pbw.tile([P, nb * 8], f16, tag="s1b")
                        with nc.allow_low_precision(reason="fp16 logits"):
                            nc.vector.tensor_reduce(
                                out=s1b[:],
                                in_=s1m[:].rearrange("p (a u) -> p a u",
                                                     u=UNITS),
                                axis=mybir.AxisListType.X,
                                op=mybir.AluOpType.add)

                    if CLEVEL >= 3:
                        # logits + leaky-relu + exp, chunk-wide
                        lg = pbw.tile([P, nb * 8], f16, tag="lg")
                        nc.vector.tensor_tensor(
                            out=lg[:], in0=s1b[:], in1=s2str[:],
                            op=mybir.AluOpType.add)
                        lgm = pbw.tile([P, nb * 8], f16, tag="lgm")
                        nc.vector.tensor_tensor(
                            out=lgm[:], in0=lg[:],
                            in1=mask_sb[:, (S + nwin) * 8:(S + nbt) * 8],
                            op=mybir.AluOpType.add)
                        neg = pbw.tile([P, nb * 8], f16, tag="neg")
                        nc.vector.tensor_scalar(
                            out=neg[:], in0=lgm[:], scalar1=0.0,
                            scalar2=LEAKY_ALPHA, op0=mybir.AluOpType.min,
                            op1=mybir.AluOpType.mult)
                        lr = pbw.tile([P, nb * 8], f16, tag="lr")
                        nc.vector.scalar_tensor_tensor(
                            out=lr[:], in0=lgm[:], scalar=0.0, in1=neg[:],
                            op0=mybir.AluOpType.max, op1=mybir.AluOpType.add)
                        expo = pbw.tile([P, nb * 8], f16, tag="expo")
                        nc.scalar.activation(
                            out=expo[:], in_=lr[:],
                            func=mybir.ActivationFunctionType.Exp,
                            bias=cbias[:])

                    if CLEVEL >= 4:
                        # expand exp over units, weight sender features
                        e128 = pbw.tile([P, nb * PAIRC], f16, tag="e128")
                        nc.vector.tensor_copy(
                            out=e128[:].rearrange("p (a u) -> p a u", u=UNITS),
                            in_=expo[:].unsqueeze(2).broadcast_to(
                                [P, nb * 8, UNITS]))
                        rhs = pbw.tile([P, nb * PAIRC], f16, tag="rhs")
                        nc.vector.tensor_tensor(
                            out=rhs[:], in0=hs[:, nwin * PAIRC:nbt * PAIRC],
                            in1=e128[:], op=mybir.AluOpType.mult)

                    if CLEVEL >= 5:
                        # per-window segmented reduces
                        off = 0
                        for i in range(nwin):
                            k = K[w0 + i]
                            if k == 0:
                                if CLEVEL >= 6:
                                    nc.gpsimd.memset(osb3[:, i, :], 0.0)
                                continue
                            rw4 = rhs[:, off * PAIRC:(off + k) * PAIRC
                                      ].rearrange("p (j t c) -> p j t c",
                                                  t=2, c=HU)
                            fold = pbw.tile([P, k * HU], f16, tag="fold")
                            nc.vector.tensor_tensor(
                                out=fold[:].rearrange("p (j c) -> p j c",
                                                      c=HU),
                                in0=rw4[:, :, 0, :], in1=rw4[:, :, 1, :],
                                op=mybir.AluOpType.add)
                            num = pbw.tile([P, HU], f32, tag="num")
                            nc.vector.tensor_reduce(
                                out=num[:],
                                in_=fold[:].rearrange("p (j c) -> p c j",
                                                      c=HU),
                                axis=mybir.AxisListType.X,
                                op=mybir.AluOpType.add)
                            den = pbw.tile([P, HEADS], f32, tag="den")
                            nc.vector.tensor_reduce(
                                out=den[:],
                                in_=expo[:, off * 8:(off + k) * 8].rearrange(
                                    "p (a h) -> p h a", h=HEADS),
                                axis=mybir.AxisListType.X,
                                op=mybir.AluOpType.add)
                            if CLEVEL >= 6:
                                dene = pbw.tile([P, HEADS], f32, tag="dene")
                                nc.vector.tensor_scalar_add(
                                    out=dene[:], in0=den[:], scalar1=1e-30)
                                rec = pbw.tile([P, HEADS], f32, tag="rec")
                                nc.vector.reciprocal(out=rec[:], in_=dene[:])
                                nc.vector.tensor_tensor(
                                    out=osb3[:, i, :].rearrange(
                                        "p (h u) -> p h u", u=UNITS),
                                    in0=num[:].rearrange("p (h u) -> p h u",
                                                         u=UNITS),
                                    in1=rec[:].unsqueeze(2).broadcast_to(
                                        [P, HEADS, UNITS]),
                                    op=mybir.AluOpType.mult)
                            off += k
                    if CLEVEL >= 6:
                        nc.sync.dma_start(
                            out=out_d[w0 * P:(w0 + nwin) * P, :].rearrange(
                                "(i p) c -> p i c", p=P),
                            in_=osb3)

    nc.compile()
    return nc


def _run(nc, per_core, n_cores):
    from concourse import bass_utils

    want_trace = bool(os.environ.get("GAT_TRACE"))
    res = bass_utils.run_bass_kernel_spmd(
        nc, per_core, core_ids=list(range(n_cores)), trace=want_trace)
    return res


def _unshard(host, results, n_cores):
    n_nodes = host["n_nodes"]
    out = np.zeros((n_nodes, HU), dtype=np.float32)
    for c in range(n_cores):
        staged = results[c]["staged"]  # [nw*128, 64]
        wn = host["win_nodes"][c]      # [nw, 128]
        valid = wn >= 0
        out[wn[valid]] = staged.reshape(wn.shape[0], P, HU)[valid]
    return out


def kernel(x, edge_index, W, att_w1, att_w2, n_cores=8, _return_results=False):
    x = np.asarray(x)
    edge_index = np.asarray(edge_index)
    W = np.asarray(W).astype(np.float32)
    att_w1 = np.asarray(att_w1).astype(np.float32)
    att_w2 = np.asarray(att_w2).astype(np.float32)

    host, per_core = _build_host_data(x, edge_index, W, att_w1, att_w2, n_cores)
    nc = _build_bass(host["plan"])
    res = _run(nc, per_core, n_cores)
    out = _unshard(host, res.results, n_cores)
    if _return_results:
        return out, res
    return out


# revision 19
# speedup vs baseline: 105.4523x; 105.4523x over previous
"""GAT (graph attention) Bass kernel for Trainium2, 8-core SPMD — v5.

Strategy: receiver-per-partition windows + batched dma_gather + chunk-wide
equal-K DVE compute.

Key HW facts driving the design (all measured on this device):
  - Any per-row DMA op (indirect_dma_start, dma_gather, scatter-add) costs
    ~9 ns/row of Q7 SWDGE descriptor generation; two SWDGE queues run on
    two Q7 cores in parallel -> ~4.5 ns/row. This is phase B's floor.
  - dma_gather moves 256B rows with the SAME descriptor count regardless
    of payload, so the pair-row table (2 nodes per 256B row) makes the 2x
    read amplification free. int16 gather indices cap the table at 32767
    rows: 25088 pairs + 1 zero sentinel fits.
  - A single dma_gather call with single_packet=True wedges the device
    above 1024 indices; single_packet=False is good to ~9216.
  - DVE tensor_copy / tensor_scalar in 2-port perf mode FULLY BLOCK the
    GpSimd SWDGE descriptor generator (shared SBUF ports). Phase B
    therefore uses only tensor_tensor / tensor_reduce / ACT for bulk work
    (leaky-relu = max(x, alpha*x), exp-broadcast fused into the weighting
    multiply), so DVE compute overlaps the gather stream.
  - Per-window ops serialize on cross-engine sem waits (~1-3 us each), so
    all windows of a chunk are padded to the chunk max degree Kc and every
    bulk op + both segmented reductions + the normalize run CHUNK-WIDE.

Per core:
  phase A: tab2[j] = [h(2j)(64) | h(2j+1)(64)] fp16 pair rows written via
           block-permuted contiguous stores; zero sentinel row at `pairs`.
  phase B: per chunk (<=WCAP windows, nwin*Kc<=BCAP sender blocks): one
           dma_gather (queue alternates between the 2 SWDGE contexts)
           fetches nwin receiver blocks + nwin*Kc sender blocks; scores
           for BOTH pair halves come from a1cat/a2cat multiplies + 16-
           group reduces; a host mask stream adds 0/-100 per (slot, half)
           (wrong half and pads exp to 0 in fp16) and selects receiver
           halves; one exp; one fused weight multiply; one num reduce
           ("p (i m c) -> p i c m"), one den reduce, one normalize.

Host scatters the staged [128, 64] window outputs back to node order.
"""

import os
import sys

import numpy as np

for _p in ("/opt/trn_rl_repo", os.path.expanduser("~/.axon_site/_ro/trn_rl_repo")):
    if os.path.isdir(_p) and _p not in sys.path:
        sys.path.insert(0, _p)

P = 128
XTILE = 1024                 # phase-A node super-tile
NBLK = XTILE // P            # 8
HEADS = 4
UNITS = 16
HU = HEADS * UNITS           # 64
PAIRC = 2 * HU               # 128 fp16 h cols per pair row
ROWC = 256                   # full table row: 512B fp16
SC1 = 128                    # cols 128:136 = [s1A | s1B]
SC2 = 136                    # cols 136:144 = [s2A | s2B]
WC = HU + 2 * HEADS          # 72 matmul cols: [h | s1 | s2]
LEAKY_ALPHA = 0.2
CSHIFT = 3.5                 # global exp shift (softmax-invariant)
PEN = -100.0                 # per-(slot,half) penalty => exp==0 in fp16
BCAP = 48                    # max sender blocks (nwin*Kc) per chunk
WCAP = 8                     # max windows per gather chunk
QSPLIT = 2                   # SWDGE queues (2 Q7 desc-gen contexts)
ABLATE = "full"              # dev-only: "phaseA" | "nocompute"
REPS = 1                     # dev-only: replicate kernel body for timing
CLEVEL = 6                   # dev-only: compute-stage ablation (0..6)


def _pair(n):
    """Node id -> (pair row, half) in the block-permuted pair table."""
    n = np.asarray(n)
    t, r = n // XTILE, n % XTILE
    p, i = r % P, r // P
    return t * (XTILE // 2) + p * (NBLK // 2) + i // 2, i % 2


def _build_host_data(x, edge_index, W, att_w1, att_w2, n_cores):
    n_nodes, in_feat = x.shape
    snd = edge_index[:, 0].astype(np.int64)
    rcv = edge_index[:, 1].astype(np.int64)

    ntiles = -(-n_nodes // XTILE)
    npad = ntiles * XTILE
    pairs = npad // 2
    sent = pairs  # sentinel pair row (zeros)

    deg = np.bincount(rcv, minlength=n_nodes)
    active = np.nonzero(deg > 0)[0]
    order_n = active[np.argsort(deg[active], kind="stable")]

    wtot = -(-len(order_n) // P)
    nw = -(-wtot // n_cores)
    wpad = nw * n_cores
    win_nodes_g = np.full((wpad, P), -1, dtype=np.int64)
    win_nodes_g.reshape(-1)[: len(order_n)] = order_n

    deg_g = np.where(win_nodes_g >= 0, deg[win_nodes_g], 0)
    k_g = deg_g.max(axis=1)
    # per-local-window block need: max over the n_cores interleaved windows
    K = k_g.reshape(nw, n_cores).max(axis=1).astype(np.int64)
    K = np.maximum(K, 1)

    # equal-K chunking: greedy, <= WCAP windows, nwin*Kc <= BCAP where
    # Kc = running max K (K is near-sorted so padding is small)
    chunks = []  # list of (w0, nwin, Kc)
    w = 0
    while w < nw:
        w0 = w
        kc = int(K[w])
        w += 1
        while w < nw and (w - w0) < WCAP:
            nk = max(kc, int(K[w]))
            if (w - w0 + 1) * nk > BCAP:
                break
            kc = nk
            w += 1
        chunks.append((w0, w - w0, kc))

    # stream block layout: per chunk [recv nwin | senders nwin*Kc]
    rcol = np.zeros(nw, dtype=np.int64)
    scol = np.zeros(nw, dtype=np.int64)
    S = 0
    for (w0, nwin, kc) in chunks:
        for i in range(nwin):
            rcol[w0 + i] = S + i
            scol[w0 + i] = S + nwin + i * kc
        S += nwin + nwin * kc
    TB = int(S)  # total stream blocks per core

    # node -> (core, local w, partition)
    node_c = np.full(n_nodes, -1, dtype=np.int64)
    node_w = np.zeros(n_nodes, dtype=np.int64)
    node_p = np.zeros(n_nodes, dtype=np.int64)
    gwin = np.repeat(np.arange(wpad), P).reshape(wpad, P)
    valid = win_nodes_g >= 0
    vn = win_nodes_g[valid]
    node_c[vn] = gwin[valid] % n_cores
    node_w[vn] = gwin[valid] // n_cores
    node_p[vn] = np.tile(np.arange(P), wpad).reshape(wpad, P)[valid]

    # edge -> slot k within its receiver's run
    eorder = np.argsort(rcv, kind="stable")
    rs = rcv[eorder]
    ss = snd[eorder]
    starts = np.zeros(n_nodes + 1, dtype=np.int64)
    starts[1:] = np.cumsum(deg)
    k_e = np.arange(len(rs)) - starts[rs]
    pair_ss, half_ss = _pair(ss)

    xT16 = np.zeros((in_feat, npad), dtype=np.float16)
    xT16[:, :n_nodes] = np.ascontiguousarray(x.T).astype(np.float16)

    # wcat = [W | W@A1 | W@A2]: matmul emits h plus both scores per node
    A12 = np.zeros((HU, 2 * HEADS), dtype=np.float32)
    for hh in range(HEADS):
        A12[hh * UNITS:(hh + 1) * UNITS, hh] = att_w1[hh, 0]
        A12[hh * UNITS:(hh + 1) * UNITS, HEADS + hh] = att_w2[hh, 0]
    wcat = np.zeros((in_feat, WC), dtype=np.float32)
    wcat[:, :HU] = W
    wcat[:, HU:] = W @ A12
    wcat16 = wcat.astype(np.float16)

    per_core = []
    win_nodes_c_all = []
    for c in range(n_cores):
        wn = win_nodes_g[c::n_cores]  # [nw, 128]
        emask = node_c[rs] == c
        er = rs[emask]
        ew = node_w[er]
        ep = node_p[er]
        ek = k_e[emask]

        # flat stream position j = B*128 + p
        idxf = np.full(TB * P, sent, dtype=np.int32)
        half = np.zeros(TB * P, dtype=np.int64)
        kind = np.zeros(TB * P, dtype=np.int8)  # 0=pad, 1=sender, 2=recv

        j_e = (scol[ew] + ek) * P + ep
        idxf[j_e] = pair_ss[emask]
        half[j_e] = half_ss[emask]
        kind[j_e] = 1

        rvalid = wn >= 0
        pr, hr = _pair(np.maximum(wn, 0))
        j_r = rcol[:, None] * P + np.arange(P)[None, :]  # [nw, 128]
        idxf[j_r.reshape(-1)] = np.where(rvalid, pr, sent).reshape(-1)
        half[j_r.reshape(-1)] = np.where(rvalid, hr, 0).reshape(-1)
        kind[j_r.reshape(-1)] = np.where(rvalid, 2, 0).reshape(-1)

        # mask stream [TB*P, 8]: senders {0, PEN} penalties per half,
        # receivers {1, 0} selector bits per half, pads PEN.
        mask = np.zeros((TB * P, 8), dtype=np.float16)
        isA = half == 0
        s = kind == 1
        mask[s, 0:4] = np.where(isA[s, None], 0.0, PEN)
        mask[s, 4:8] = np.where(isA[s, None], PEN, 0.0)
        r = kind == 2
        mask[r, 0:4] = np.where(isA[r, None], 1.0, 0.0)
        mask[r, 4:8] = np.where(isA[r, None], 0.0, 1.0)
        pad = kind == 0
        mask[pad, :] = PEN

        # idx16: wrapped-16 layout [16, TB*8] replicated to [128, TB*8]
        idxw = idxf.astype(np.int16).reshape(TB * 8, 16).T
        idx16 = np.ascontiguousarray(np.tile(idxw, (8, 1)))

        mask16 = np.ascontiguousarray(
            mask.reshape(TB, P, 8).transpose(1, 0, 2).reshape(P, TB * 8))

        per_core.append({
            "xT16": xT16,
            "wcat": wcat16,
            "idx16": idx16,
            "mask16": mask16,
        })
        win_nodes_c_all.append(wn)

    plan = {
        "npad": npad, "ntiles": ntiles, "nw": nw, "pairs": pairs,
        "TB": TB, "chunks": chunks, "in_feat": in_feat,
    }
    host = {"plan": plan, "win_nodes": win_nodes_c_all, "n_nodes": n_nodes}
    return host, per_core


def _build_bass(plan):
    from concourse import bacc, mybir, tile
    import concourse.bass as bass

    f16 = mybir.dt.float16
    f32 = mybir.dt.float32
    i16 = mybir.dt.int16

    npad = plan["npad"]
    ntiles = plan["ntiles"]
    nw = plan["nw"]
    pairs = plan["pairs"]
    TB = plan["TB"]
    chunks = plan["chunks"]
    in_feat = plan["in_feat"]

    nc = bacc.Bacc("TRN2", target_bir_lowering=False, debug=False,
                   enable_asserts=False, num_devices=1,
                   num_swdge_queues=QSPLIT)

    xT_d = nc.dram_tensor("xT16", [in_feat, npad], f16, kind="ExternalInput").ap()
    w_d = nc.dram_tensor("wcat", [in_feat, WC], f16, kind="ExternalInput").ap()
    idx_d = nc.dram_tensor("idx16", [P, TB * 8], i16, kind="ExternalInput").ap()
    mask_d = nc.dram_tensor("mask16", [P, TB * 8], f16, kind="ExternalInput").ap()

    out_d = nc.dram_tensor("staged", [nw * P, HU], f32, kind="ExternalOutput").ap()
    tab_d = nc.dram_tensor("tab2", [pairs + 1, ROWC], f16, kind="Internal").ap()

    with tile.TileContext(nc) as tc:
        with tc.tile_pool(name="consts", bufs=1) as cpool:
            w_sb = cpool.tile([in_feat, WC], f16, tag="wcat")
            nc.sync.dma_start(out=w_sb[:], in_=w_d[:])
            idx_sb = cpool.tile([P, TB * 8], i16, tag="idx16")
            nc.sync.dma_start(out=idx_sb[:], in_=idx_d[:])
            mask_sb = cpool.tile([P, TB * 8], f16, tag="mask16")
            nc.sync.dma_start(out=mask_sb[:], in_=mask_d[:])
            zrow = cpool.tile([1, ROWC], f16, tag="zrow")
            nc.gpsimd.memset(zrow[:], 0.0)
            nc.sync.dma_start(out=tab_d[pairs:pairs + 1, :], in_=zrow[:])
            cbias = cpool.tile([P, 1], f32, tag="cbias")
            nc.gpsimd.memset(cbias[:], -CSHIFT)
            alph = cpool.tile([P, 1], f16, tag="alph")
            nc.gpsimd.memset(alph[:], LEAKY_ALPHA)
            epsc = cpool.tile([P, 1], f32, tag="epsc")
            nc.gpsimd.memset(epsc[:], 1e-30)

            # ---- phase A: pair table tab2[t*512 + p*4 + i] ----
            with tc.tile_pool(name="pa_x", bufs=3) as pax, \
                 tc.tile_pool(name="pa_ps", bufs=4, space="PSUM") as paps, \
                 tc.tile_pool(name="pa_hs", bufs=3) as pahs:
              for _rep in range(REPS):
                for t in range(ntiles):
                    xt = pax.tile([in_feat, XTILE], f16, tag="xt")
                    nc.sync.dma_start(
                        out=xt[:], in_=xT_d[:, t * XTILE:(t + 1) * XTILE])
                    # hst: 4 pair rows x 512B = [hA hB | s1A s1B | s2A s2B |..]
                    hst = pahs.tile([P, (NBLK // 2) * ROWC], f16, tag="hst")
                    nc.gpsimd.memset(
                        hst[:].rearrange("p (q c) -> p q c", c=ROWC)[
                            :, :, SC2 + 8:ROWC], 0.0)
                    half = NBLK // 2
                    for g in range(2):
                        ps = paps.tile([P, half * WC], f32, tag="ps")
                        for i in range(half):
                            b = g * half + i
                            nc.tensor.matmul(
                                out=ps[:, i * WC:(i + 1) * WC],
                                lhsT=xt[:, b * P:(b + 1) * P],
                                rhs=w_sb[:], start=True, stop=True)
                        # blocks (pair q, half e); pair col = (g*2+q)*ROWC
                        ps4 = ps[:].rearrange("p (q e c) -> p q e c",
                                              q=2, e=2)
                        h4 = hst[:].rearrange("p (q c) -> p q c", c=ROWC)
                        hbase = g * 2
                        nc.vector.tensor_copy(
                            out=h4[:, hbase:hbase + 2, 0:PAIRC].rearrange(
                                "p q (e c) -> p q e c", e=2),
                            in_=ps4[:, :, :, 0:HU])
                        nc.vector.tensor_copy(
                            out=h4[:, hbase:hbase + 2, SC1:SC1 + 8].rearrange(
                                "p q (e c) -> p q e c", e=2),
                            in_=ps4[:, :, :, HU:HU + HEADS])
                        nc.vector.tensor_copy(
                            out=h4[:, hbase:hbase + 2, SC2:SC2 + 8].rearrange(
                                "p q (e c) -> p q e c", e=2),
                            in_=ps4[:, :, :, HU + HEADS:WC])
                    nc.sync.dma_start(
                        out=tab_d[t * (XTILE // 2):(t + 1) * (XTILE // 2),
                                  :].rearrange("(p i) c -> p i c", p=P),
                        in_=hst[:].rearrange("p (i c) -> p i c", c=ROWC))

            # ---- phase B: chunked gather + chunk-wide attention ----
            if ABLATE != "phaseA":
              with tc.tile_pool(name="pb_hs", bufs=2) as pbh, \
                   tc.tile_pool(name="pb_w", bufs=2) as pbw, \
                   tc.tile_pool(name="pb_o", bufs=2) as pbo:
                for _rep in range(REPS):
                  S = 0
                  for _ci, (w0, nwin, kc) in enumerate(chunks):
                    nb = nwin * kc
                    nbt = nwin + nb
                    S_c = S
                    S += nbt
                    hs = pbh.tile([P, nbt * ROWC], f16, tag="hs")
                    hs3 = hs[:].rearrange("p (j c) -> p j c", c=ROWC)
                    nc.gpsimd.dma_gather(
                        out_ap=hs3,
                        in_ap=tab_d[:],
                        idxs_ap=idx_sb[:, S_c * 8:(S_c + nbt) * 8],
                        num_idxs=nbt * P,
                        num_idxs_reg=nbt * P,
                        elem_size=ROWC,
                        single_packet=False,
                        queue_num=_ci % QSPLIT)
                    if ABLATE == "nocompute":
                        continue

                    osb = pbo.tile([P, nwin * HU], f32, tag="osb")

                    if CLEVEL >= 1:
                        # compact gathered score cols on ACT (strided reads
                        # overlap the gather; DVE then stays flat/contiguous)
                        s2sc = pbw.tile([P, nwin * 8], f16, tag="s2sc")
                        nc.scalar.activation(
                            out=s2sc[:].rearrange("p (j q) -> p j q", q=8),
                            in_=hs3[:, 0:nwin, SC2:SC2 + 8],
                            func=mybir.ActivationFunctionType.Copy)
                        s2s = pbw.tile([P, nwin * 8], f16, tag="s2s")
                        nc.vector.tensor_tensor(
                            out=s2s[:], in0=s2sc[:],
                            in1=mask_sb[:, S_c * 8:(S_c + nwin) * 8],
                            op=mybir.AluOpType.mult)
                        s2s3 = s2s[:].rearrange("p (i t h) -> p i t h",
                                                t=2, h=HEADS)
                        s2f = pbw.tile([P, nwin * HEADS], f16, tag="s2f")
                        nc.vector.tensor_tensor(
                            out=s2f[:].rearrange("p (i h) -> p i h", h=HEADS),
                            in0=s2s3[:, :, 0, :], in1=s2s3[:, :, 1, :],
                            op=mybir.AluOpType.add)
                        # s2str[p, (i j t h)] = s2f[p, (i h)] bcast over
                        # (j t) -- on ACT so DVE never enters a copy (2-port)
                        # mode that would block SWDGE desc-gen
                        s2str = pbw.tile([P, nb * 8], f16, tag="s2str")
                        nc.scalar.activation(
                            out=s2str[:].rearrange("p (i a h) -> p i a h",
                                                   a=2 * kc, h=HEADS),
                            in_=s2f[:].rearrange("p (i h) -> p i h",
                                                 h=HEADS).unsqueeze(
                                2).broadcast_to([P, nwin, 2 * kc, HEADS]),
                            func=mybir.ActivationFunctionType.Copy)

                    if CLEVEL >= 3:
                        # compact sender s1 cols on ACT, then flat logit adds
                        sc1c = pbw.tile([P, nb * 8], f16, tag="sc1c")
                        nc.scalar.activation(
                            out=sc1c[:].rearrange("p (j q) -> p j q", q=8),
                            in_=hs3[:, nwin:nbt, SC1:SC1 + 8],
                            func=mybir.ActivationFunctionType.Copy)
                        lg = pbw.tile([P, nb * 8], f16, tag="lg")
                        nc.vector.tensor_tensor(
                            out=lg[:], in0=sc1c[:], in1=s2str[:],
                            op=mybir.AluOpType.add)
                        lgm = pbw.tile([P, nb * 8], f16, tag="lgm")
                        nc.vector.tensor_tensor(
                            out=lgm[:], in0=lg[:],
                            in1=mask_sb[:, (S_c + nwin) * 8:(S_c + nbt) * 8],
                            op=mybir.AluOpType.add)
                        ax = pbw.tile([P, nb * 8], f16, tag="ax")
                        nc.vector.tensor_tensor(
                            out=ax[:], in0=lgm[:],
                            in1=alph[:].broadcast_to([P, nb * 8]),
                            op=mybir.AluOpType.mult)
                        lr = pbw.tile([P, nb * 8], f16, tag="lr")
                        nc.vector.tensor_tensor(
                            out=lr[:], in0=lgm[:], in1=ax[:],
                            op=mybir.AluOpType.max)
                        expo = pbw.tile([P, nb * 8], f16, tag="expo")
                        nc.scalar.activation(
                            out=expo[:], in_=lr[:],
                            func=mybir.ActivationFunctionType.Exp,
                            bias=cbias[:])

                    if CLEVEL >= 4:
                        # ACT writes the unit-expanded exp directly; DVE then
                        # does one flat 2x multiply (no copy-mode DVE ops)
                        e128 = pbw.tile([P, nb * PAIRC], f16, tag="e128")
                        nc.scalar.activation(
                            out=e128[:].rearrange("p (a u) -> p a u", u=UNITS),
                            in_=lr[:].unsqueeze(2).broadcast_to(
                                [P, nb * 8, UNITS]),
                            func=mybir.ActivationFunctionType.Exp,
                            bias=cbias[:])
                        rhs = pbw.tile([P, nb * PAIRC], f16, tag="rhs")
                        nc.vector.tensor_tensor(
                            out=rhs[:].rearrange("p (j c) -> p j c", c=PAIRC),
                            in0=e128[:].rearrange("p (j c) -> p j c", c=PAIRC),
                            in1=hs3[:, nwin:nbt, 0:PAIRC],
                            op=mybir.AluOpType.mult)

                    if CLEVEL >= 5:
                        # chunk-wide segmented reductions over m=(j,t) via
                        # binary-tree folds (contiguous-innermost 4D adds)
                        def _tree(src_ap, m, c, tag):
                            lvl = 0
                            cur = src_ap  # AP, cols (i, m, c)
                            while m > 1:
                                m2 = m // 2
                                odd = m - 2 * m2
                                dt = f32 if m2 == 1 else f16
                                dst = pbw.tile([P, nwin * m2 * c], dt,
                                               tag=f"{tag}{lvl}")
                                v = cur.rearrange("p (i m c) -> p i m c",
                                                  m=m, c=c)
                                d3 = dst[:].rearrange("p (i m c) -> p i m c",
                                                      m=m2, c=c)
                                nc.vector.tensor_tensor(
                                    out=d3, in0=v[:, :, 0:m2, :],
                                    in1=v[:, :, m2:2 * m2, :],
                                    op=mybir.AluOpType.add)
                                if odd:
                                    nc.vector.tensor_tensor(
                                        out=d3[:, :, 0, :],
                                        in0=d3[:, :, 0, :],
                                        in1=v[:, :, 2 * m2, :],
                                        op=mybir.AluOpType.add)
                                cur = dst[:]
                                m = m2
                                lvl += 1
                            return cur  # [P, nwin*c] f32

                        num = _tree(rhs[:], 2 * kc, HU, "numt")
                        den = _tree(expo[:], 2 * kc, HEADS, "dent")

                    if CLEVEL >= 6:
                        dene = pbw.tile([P, nwin * HEADS], f32, tag="dene")
                        nc.vector.tensor_tensor(
                            out=dene[:], in0=den,
                            in1=epsc[:].broadcast_to([P, nwin * HEADS]),
                            op=mybir.AluOpType.add)
                        rec = pbw.tile([P, nwin * HEADS], f32, tag="rec")
                        nc.vector.reciprocal(out=rec[:], in_=dene[:])
                        # unit-expand 1/den on ACT, then one flat f32 mult
                        recE = pbw.tile([P, nwin * HU], f32, tag="recE")
                        nc.scalar.activation(
                            out=recE[:].rearrange("p (a u) -> p a u", u=UNITS),
                            in_=rec[:].unsqueeze(2).broadcast_to(
                                [P, nwin * HEADS, UNITS]),
                            func=mybir.ActivationFunctionType.Copy)
                        nc.vector.tensor_tensor(
                            out=osb[:], in0=num, in1=recE[:],
                            op=mybir.AluOpType.mult)
                        nc.sync.dma_start(
                            out=out_d[w0 * P:(w0 + nwin) * P, :].rearrange(
                                "(i p) c -> p i c", p=P),
                            in_=osb[:].rearrange("p (i c) -> p i c", c=HU))

    nc.compile()
    return nc


def _run(nc, per_core, n_cores):
    from concourse import bass_utils

    want_trace = bool(os.environ.get("GAT_TRACE"))
    res = bass_utils.run_bass_kernel_spmd(
        nc, per_core, core_ids=list(range(n_cores)), trace=want_trace)
    return res


def _unshard(host, results, n_cores):
    n_nodes = host["n_nodes"]
    out = np.zeros((n_nodes, HU), dtype=np.float32)
    for c in range(n_cores):
        staged = results[c]["staged"]  # [nw*128, 64]
        wn = host["win_nodes"][c]      # [nw, 128]
        valid = wn >= 0
        out[wn[valid]] = staged.reshape(wn.shape[0], P, HU)[valid]
    return out


def kernel(x, edge_index, W, att_w1, att_w2, n_cores=8, _return_results=False):
    x = np.asarray(x)
    edge_index = np.asarray(edge_index)
    W = np.asarray(W).astype(np.float32)
    att_w1 = np.asarray(att_w1).astype(np.float32)
    att_w2 = np.asarray(att_w2).astype(np.float32)

    host, per_core = _build_host_data(x, edge_index, W, att_w1, att_w2, n_cores)
    nc = _build_bass(host["plan"])
    res = _run(nc, per_core, n_cores)
    out = _unshard(host, res.results, n_cores)
    if _return_results:
        return out, res
    return out
